# revision 1
# baseline (speedup 1.0000x reference)
"""Trainium2 Bass kernel for nn_RNN_6167573037204.

LSTM (input=1, hidden=24, T=1024) over batch 2048 + tiny MLP head.
Data-parallel: batch sharded 8 ways (256 per core); weights replicated.

Per-core layout: hidden dim on partitions, batch on the free dim.
Gates are computed with a single matmul group into PSUM
(x-projection K=1 + h-projection K=24, accumulated), then ONE tanh
activation covers all four gates using a per-partition scale
(sigmoid(x) = (tanh(x/2)+1)/2), with cell/hidden state kept doubled
(C=2c, H=2h) so the affine fixups fold into fused scalar_tensor_tensor
DVE ops and into the matmul weights:

Hardware constraints honored here:
 - compute-engine access patterns must start at partition 0/32/64/96
 - a 2-input DVE op with both inputs in SBUF needs EQUAL start partitions

So the gate blocks are padded to 32 partitions in PSUM (i@0:24, f@32:56,
g@64:88, o@96:120, zero weight columns in the pads) and every DVE operand
pair sits at matching partition offsets:

  z  = s_k*(W x + W_h h + b)  per gate   (PSUM [128,n], 2-matmul group:
       MM_x: lhsT=[W_ih;b] gate-scaled, rhs=[x_t;1] (K=2, start=True);
       MM_h: lhsT=W_hh^T/2 gate-scaled, rhs=H (K=24, accumulate))
  GA = tanh(z[0:56])                      (ACT_a: ti@0:24, tf@32:56)
  GB = tanh(z[64:120])                    (ACT_b: tg@0:24, to@32:56)
  PA = (GA[0:24] + 1) * GB[0:24]          (STT1a @0:  (ti+1)*tg)
  PB = (GA[32:56] + 1) * CT[32:56]        (STT1b @32: (tf+1)*C)
  CTnext[32:56] = 0.5*PB + PA             (STT2 @0)
  TC[32:56] = tanh(0.5*CTnext[32:56])     (ACT3)
  H_next = (GB[32:56] + 1) * TC[32:56]    (STT3 @32 -> HXnext[0:24])

Gate scales s_k (0.5 for i,f,o sigmoid-via-tanh; 1.0 for g) and biases are
folded into the matmul weights + a ones-row, because activation with
per-partition scale/bias APs reading PSUM fails at runtime on this HW.
C (doubled cell state) persists in rows 32:56 of ping-pong CT tiles. The
[x_t; 1] pair is staged per step into a [2, ns] tile by a small
SBUF->SBUF DMA (DMA is exempt from the partition rules). After Tile
scheduling, _strip_redundant_matmul_waits caps every instruction at the
1-sync-wait HW limit using engine-order transitivity arguments.
"""

import numpy as np

H = 24
B = 2048
T = 1024
NCORES = 8
N = B // NCORES  # 256 batch columns per core

# torch gate order (i,f,g,o) -> kernel order (i,f,o,g)
_PERM = np.concatenate([np.arange(0, 48), np.arange(72, 96), np.arange(48, 72)])

_NC_CACHE = {}


def _build_nc(t_steps=T, n=N, n_streams=1, gpsimd_ops=()):
    """n_streams independent batch-chunk recurrence chains are interleaved to
    hide the serial per-step latency; gpsimd_ops names STT ops ("PA","PB","H")
    to offload from DVE to the GPSIMD engine."""
    import concourse.bass as bass
    import concourse.mybir as mybir
    import concourse.tile as tile
    from concourse.tile import add_dep_helper
    from contextlib import ExitStack

    f32 = mybir.dt.float32
    AF = mybir.ActivationFunctionType
    ALU = mybir.AluOpType

    assert n % n_streams == 0
    ns = n // n_streams
    nblk = (t_steps + 63) // 64

    nc = bass.Bass()
    # single packed constant block -> one DMA (avoids many-sem fan-in on the
    # first instructions; the HW has a small per-instruction sync-wait limit):
    # cols 0:128 rows 0:24 lhsT_h (gate-scaled); cols 128:256 rows 0:2
    # lhsT_x2 ([W_ih; b] gate-scaled); cols 260:284 rows 0:24 w1T/2;
    # cols 284:308 row 0 b1 (as lhsT); col 308 rows 0:24 w2T; col 309 b2;
    # cols 312:312+n rows 0:24 H0 (doubled), rows 32:56 C0 (doubled);
    # cols 312+n:312+2n rows 0:2 [x_0; 1]
    CW = 312 + 2 * n
    d_cp = nc.declare_dram_parameter("const_pack", [128, CW], f32, isOutput=False)
    d_xT = nc.declare_dram_parameter("xT", [128, nblk * n], f32, isOutput=False)
    d_out = nc.declare_dram_parameter("out", [1, n], f32, isOutput=True)

    with ExitStack() as ctx:
        tc = ctx.enter_context(tile.TileContext(nc))
        consts = ctx.enter_context(tc.tile_pool(name="consts", bufs=1))
        psum_bufs = max(2, 4 // n_streams)
        psum_pool = ctx.enter_context(
            tc.tile_pool(name="psum", bufs=psum_bufs, space="PSUM")
        )
        p_pool = ctx.enter_context(tc.tile_pool(name="prod", bufs=3))
        tc_pool = ctx.enter_context(tc.tile_pool(name="tanh_c", bufs=3))
        h_pool = ctx.enter_context(tc.tile_pool(name="hstate", bufs=3))
        xs_pool = ctx.enter_context(tc.tile_pool(name="xstage", bufs=3))

        # constants / persistent state
        xt = consts.tile([128, nblk * n], f32)
        nc.sync.dma_start(xt[:, :], d_xT[:, :])
        cp = consts.tile([128, CW], f32)
        nc.sync.dma_start(cp[:, :], d_cp[:, :])
        # weights staged via DVE copies so first-step matmuls wait on ONE proc
        lhx_h = consts.tile([H, 128], f32)
        nc.vector.tensor_copy(lhx_h[:, :], cp[0:H, 0:128])
        lhx_x = consts.tile([2, 128], f32)
        nc.vector.tensor_copy(lhx_x[:, :], cp[0:2, 128:256])
        w1 = consts.tile([H, H], f32)
        nc.vector.tensor_copy(w1[:, :], cp[0:H, 260:284])
        b1r = consts.tile([1, H], f32)
        nc.vector.tensor_copy(b1r[:, :], cp[0:1, 284:308])
        w2 = consts.tile([H, 1], f32)
        nc.vector.tensor_copy(w2[:, :], cp[0:H, 308:309])
        b2r = consts.tile([1, 1], f32)
        nc.vector.tensor_copy(b2r[:, :], cp[0:1, 309:310])
        ones = consts.tile([1, ns], f32)
        # ones row: host stores 1.0 in row 1 of the [x0; 1] block; a
        # row-1 source is partition offset 1 (illegal for compute engines),
        # so host also writes ones at row 0, cols 310:311 extended block;
        # simplest: copy the x0-pair tile's row 1 via DMA (exempt)
        nc.sync.dma_start(ones[:, :], d_cp[1:2, 312 + n : 312 + n + ns])

        # per-stream ping-pong cell-state tiles; C (doubled) lives in rows 32:56
        CT = []
        for s in range(n_streams):
            c0 = consts.tile([56, ns], f32, tag=f"CT0_{s}")
            c1 = consts.tile([56, ns], f32, tag=f"CT1_{s}")
            nc.vector.tensor_copy(
                c0[32:56, :], cp[32:56, 312 + s * ns : 312 + (s + 1) * ns]
            )
            CT.append((c0, c1))

        # [24, ns] doubled-hidden tiles + per-step [1, ns] x staging tiles
        HX = []
        XS = []
        for s in range(n_streams):
            hx = h_pool.tile([H, ns], f32, tag=f"HX_{s}")
            nc.vector.tensor_copy(
                hx[:, :], cp[0:H, 312 + s * ns : 312 + (s + 1) * ns]
            )
            HX.append(hx)
            x0 = xs_pool.tile([2, ns], f32, tag=f"XS_{s}")
            nc.vector.tensor_copy(
                x0[:, :], cp[0:2, 312 + n + s * ns : 312 + n + (s + 1) * ns]
            )
            XS.append(x0)

        xdma_ring = []
        dve_or_gp = {
            name: (nc.gpsimd if name in gpsimd_ops else nc.vector)
            for name in ("PA", "PB", "C", "H")
        }

        for t in range(t_steps):
            for s in range(n_streams):
                cols = slice(s * ns, (s + 1) * ns)
                CTc = CT[s][t % 2]
                CTn = CT[s][1 - t % 2]
                # stage next step's x into its own [1, ns] tile (DMA; off
                # critical path) so each matmul carries exactly one sync wait
                HXn = h_pool.tile([H, ns], f32, tag=f"HX_{s}")
                if t + 1 < t_steps:
                    blk, row = divmod(t + 1, 64)
                    xsn = xs_pool.tile([2, ns], f32, tag=f"XS_{s}")
                    xd = nc.sync.dma_start(
                        xsn[:, :],
                        xt[
                            2 * row : 2 * row + 2,
                            blk * n + s * ns : blk * n + (s + 1) * ns,
                        ],
                    )
                    xdma_ring.append(xd)
                    if len(xdma_ring) > 8:
                        xdma_ring.pop(0)
                # two base-0 psum tiles (i,f) and (g,o): both gate ACTs
                # read PSUM at partition 0 (offset-64 PSUM ACT reads are
                # suspect at runtime, like the AP-scale ACT bug)
                psA = psum_pool.tile([56, ns], f32, tag=f"psA_{s}")
                nc.tensor.matmul(
                    psA[:, :], lhx_x[:, 0:56], XS[s][:, :], start=True, stop=False
                )
                nc.tensor.matmul(
                    psA[:, :], lhx_h[:, 0:56], HX[s][:, :], start=False, stop=True
                )
                psB = psum_pool.tile([56, ns], f32, tag=f"psB_{s}")
                nc.tensor.matmul(
                    psB[:, :], lhx_x[:, 64:120], XS[s][:, :], start=True, stop=False
                )
                nc.tensor.matmul(
                    psB[:, :], lhx_h[:, 64:120], HX[s][:, :], start=False, stop=True
                )
                GA = p_pool.tile([56, ns], f32, tag=f"GA_{s}")
                nc.scalar.activation(GA[:, :], psA[:, :], AF.Tanh)
                GB = p_pool.tile([56, ns], f32, tag=f"GB_{s}")
                nc.scalar.activation(GB[:, :], psB[:, :], AF.Tanh)
                PA = p_pool.tile([H, ns], f32, tag=f"PA_{s}")
                dve_or_gp["PA"].scalar_tensor_tensor(
                    PA[:, :], GA[0:H, :], 1.0, GB[0:H, :], op0=ALU.add, op1=ALU.mult
                )
                PB = p_pool.tile([H, ns], f32, tag=f"PB_{s}")
                dve_or_gp["PB"].scalar_tensor_tensor(
                    PB[:, :], GA[32:56, :], 1.0, CTc[32:56, :],
                    op0=ALU.add, op1=ALU.mult,
                )
                dve_or_gp["C"].scalar_tensor_tensor(
                    CTn[32:56, :], PB[:, :], 0.5, PA[:, :], op0=ALU.mult, op1=ALU.add
                )
                TC = tc_pool.tile([56, ns], f32, tag=f"TC_{s}")
                nc.scalar.activation(TC[32:56, :], CTn[32:56, :], AF.Tanh, scale=0.5)
                dve_or_gp["H"].scalar_tensor_tensor(
                    HXn[:, :], GB[32:56, :], 1.0, TC[32:56, :],
                    op0=ALU.add, op1=ALU.mult,
                )
                HX[s] = HXn
                if t + 1 < t_steps:
                    XS[s] = xsn

        # MLP head: z1 = relu(W1 h + b1); out = relu(W2 z1 + b2)
        protected_mms = set()
        fence_targets = list(xdma_ring)
        for s in range(n_streams):
            ps1 = psum_pool.tile([H, ns], f32, tag=f"psA_{s}")
            nc.tensor.matmul(ps1[:, :], w1[:, :], HX[s][:, :], start=True, stop=False)
            nc.tensor.matmul(ps1[:, :], b1r[:, :], ones[:, :], start=False, stop=True)
            z1 = p_pool.tile([H, ns], f32, tag=f"z1_{s}")
            nc.scalar.activation(z1[:, :], ps1[:, :], AF.Relu)
            ps2 = psum_pool.tile([1, ns], f32, tag=f"psB_{s}")
            mm2 = nc.tensor.matmul(
                ps2[:, :], w2[:, :], z1[:, :], start=True, stop=False
            )
            protected_mms.add(mm2.ins.name)
            nc.tensor.matmul(ps2[:, :], b2r[:, :], ones[:, :], start=False, stop=True)
            o_t = tc_pool.tile([1, ns], f32, tag=f"ot_{s}")
            oact = nc.scalar.activation(o_t[:, :], ps2[:, :], AF.Relu)
            odma = nc.sync.dma_start(d_out[0:1, s * ns : (s + 1) * ns], o_t[:, :])
            fence_targets += [mm2, oact, odma]

        # fence chain: one single-wait DVE copy per kernel-final instruction;
        # the tail drain then only needs its DVE wait (the rest are stripped)
        fdummy = consts.tile([1, 32], f32)
        for k, tgt in enumerate(fence_targets):
            cop = nc.vector.tensor_copy(fdummy[0:1, k : k + 1], cp[0:1, k : k + 1])
            add_dep_helper(cop.ins, tgt.ins, sync=True, reason="drain fence")

    _strip_redundant_matmul_waits(nc, protected_mms)
    return nc


def _strip_redundant_matmul_waits(nc, protected_mms=()):
    """Matmult instructions have only ONE HW sync-wait slot; Tile emits up to
    4 waits (RAW on its rhs producer, plus psum-slot-reuse WAR vs the gate
    ACTs and WAW vs the old matmul, all ~8 steps stale). The reuse waits are
    redundant here: PE executes matmuls in order (so PE self-waits are
    implied), and any in-order PE predecessor's DVE wait on STT3(t-1)
    transitively implies the 8-step-old gate ACTs finished (STT3(t-1) waits
    on ACT3(t-1); the ACT engine executes in order; ACT(t-8) precedes
    ACT3(t-1) in any topological order of the recurrence). Keep only the RAW
    wait (DVE / DMA / Pool). The final MLP matmul reads an ACT output, so
    its Activation wait is genuine and protected."""
    for blk in nc.m.functions[0].blocks:
        for inst in blk.instructions:
            ty = type(inst).__name__
            si = inst.sync_info
            if si is None or not si.on_wait:
                continue
            if ty == "InstDrain" and len(si.on_wait) > 1:
                # the fence chain (DVE) transitively implies every other proc
                kept = [c for c in si.on_wait if c.ant_name.startswith("DVE")]
                assert len(kept) == 1, [c.ant_name for c in si.on_wait]
                si.on_wait = kept
                continue
            if ty not in ("InstMatmult", "InstDMACopy", "InstActivation",
                          "InstTensorScalarPtr", "InstTensorCopy"):
                continue
            if len(si.on_wait) <= 1:
                continue
            own = None
            if si.on_update:
                own = si.on_update[0].ant_name.split("_")[0]
            kept = list(si.on_wait)
            # same-engine waits: engines issue in order and every DVE op ends
            # in a pipeline DRAIN, so same-engine RAW/WAR/WAW is implied
            if own and len(kept) > 1:
                kept = [c for c in kept if c.ant_name.split("_")[0] != own]
            if ty == "InstMatmult" and inst.name not in protected_mms:
                # psum-slot WAR vs the 8-step-old gate ACTs: implied by this
                # engine's earlier DVE wait on STT3(t-1) (ACT runs in order)
                kept = [c for c in kept if not c.ant_name.startswith("Activation")]
            elif ty == "InstDMACopy":
                # x-staging DMA: the PE wait (matmul that read the slot being
                # overwritten) implies the slot's previous writer finished
                if any(c.ant_name.startswith("PE") for c in kept):
                    kept = [c for c in kept if c.ant_name.startswith("PE")]
            elif ty == "InstActivation":
                # first gate ACT: the const-pack DMA is implied by the PE wait
                # (the matmul waited on the DVE weight-copies that read it)
                if any(c.ant_name.startswith("PE") for c in kept):
                    kept = [c for c in kept if not c.ant_name.startswith("DMAHW")]
            elif ty == "InstTensorScalarPtr":
                # STT3's HX-slot WAR vs MM(t-3): implied by its Activation wait
                # (ACT3(t) -> ... -> ACT_a(t) waited on MM(t); engines in-order)
                if any(c.ant_name.startswith("Activation") for c in kept):
                    kept = [c for c in kept if not c.ant_name.startswith("PE")]
            assert len(kept) <= 1, (ty, inst.name,
                                    [c.ant_name for c in si.on_wait])
            si.on_wait = kept


def _prep_core_inputs(x, h_state, c_state, W_ih, W_hh, b_ih, b_hh, W1, b1, W2, b2,
                      t_steps=T, n=N):
    """Host-side prep: shard + transpose + fold constants. Returns in_maps."""
    nblk = (t_steps + 63) // 64
    b = (b_ih + b_hh).astype(np.float64)
    # torch gate chunks: i=0:24, f=24:48, g=48:72, o=72:96
    # padded psum layout: i@0:24, f@32:56, g@64:88, o@96:120
    # per-gate tanh scales (i,f,o: 0.5 for sigmoid-via-tanh; g: 1.0) are
    # folded into the weights; gate biases ride a ones-row in lhsT_x2
    lhsT_h = np.zeros((H, 128), np.float32)
    lhsT_x = np.zeros((2, 128), np.float32)
    gate_s = {0: 0.5, 1: 0.5, 2: 1.0, 3: 0.5}  # psum blocks i,f,g,o
    blk_gate = [0, 1, 3, 2]  # torch chunk index per psum block (i,f,g,o order)
    for kb, blk_lo in enumerate([0, 32, 64, 96]):
        k = blk_gate[kb] if False else kb
        pass
    for kb, blk_lo in enumerate([0, 32, 64, 96]):
        gsl = slice(24 * kb, 24 * (kb + 1))
        sc = 0.5 if kb != 2 else 1.0
        lhsT_h[:, blk_lo : blk_lo + 24] = W_hh[gsl, :].T / 2.0 * sc
        lhsT_x[0, blk_lo : blk_lo + 24] = W_ih[gsl, 0] * sc
        lhsT_x[1, blk_lo : blk_lo + 24] = b[gsl] * sc
    # const_pack layout (see _build_nc)
    CW = 312 + 2 * n
    cp = np.zeros((128, CW), np.float32)
    cp[0:H, 0:128] = lhsT_h
    cp[0:2, 128:256] = lhsT_x
    cp[0:H, 260:284] = W1.T / 2.0
    cp[0, 284:308] = b1
    cp[0:H, 308] = W2[0, :]
    cp[0, 309] = b2[0]

    in_maps = []
    for c in range(NCORES):
        sl = slice(c * n, (c + 1) * n)
        xs = x[sl, :t_steps, 0].astype(np.float32)  # [n, t]
        pad_t = nblk * 64 - t_steps
        if pad_t:
            xs = np.concatenate([xs, np.zeros((n, pad_t), np.float32)], axis=1)
        # row 2k = x_{blk*64+k}, row 2k+1 = ones
        xpair = np.ones((nblk * 64, 2, n), np.float32)
        xpair[:, 0, :] = xs.T
        xT = np.ascontiguousarray(
            xpair.reshape(nblk, 64 * 2, n).transpose(1, 0, 2).reshape(128, nblk * n)
        )
        cpc = cp.copy()
        cpc[0:H, 312 : 312 + n] = 2.0 * h_state[0, sl, :].T
        cpc[32:56, 312 : 312 + n] = 2.0 * c_state[0, sl, :].T
        cpc[0, 312 + n : 312 + 2 * n] = xs[:, 0]
        cpc[1, 312 + n : 312 + 2 * n] = 1.0
        in_maps.append({"const_pack": cpc, "xT": xT})
    return in_maps


# tuned configuration for the final kernel
_CONFIG = {"n_streams": 1, "gpsimd_ops": ()}


def _run(in_maps, t_steps=T, n=N, trace=False, **kw):
    from concourse.bass_utils import run_bass_kernel_spmd

    key = (t_steps, n, _CONFIG["n_streams"], tuple(_CONFIG["gpsimd_ops"]))
    if key not in _NC_CACHE:
        _NC_CACHE[key] = _build_nc(
            t_steps, n, n_streams=_CONFIG["n_streams"],
            gpsimd_ops=tuple(_CONFIG["gpsimd_ops"]),
        )
    nc = _NC_CACHE[key]
    return run_bass_kernel_spmd(nc, in_maps, list(range(NCORES)), trace=trace, **kw)


def kernel(x, h_state, c_state, y, W_ih, W_hh, b_ih, b_hh, W1, b1, W2, b2):
    x = np.asarray(x); h_state = np.asarray(h_state); c_state = np.asarray(c_state)
    in_maps = _prep_core_inputs(
        x, h_state, c_state,
        np.asarray(W_ih), np.asarray(W_hh), np.asarray(b_ih), np.asarray(b_hh),
        np.asarray(W1), np.asarray(b1), np.asarray(W2), np.asarray(b2),
    )
    res = _run(in_maps)
    out = np.concatenate([res.results[c]["out"][0] for c in range(NCORES)])
    return out.reshape(1, B, 1).astype(np.float32)

